# revision 4
# baseline (speedup 1.0000x reference)
"""GAT layer (nn_GATLayer_32719060861314) as a Bass/Tile SPMD kernel on 8 trn2 cores.

Strategy (edge-sharded, dst-partitioned, gather-free):
  - Node dsts are range-partitioned over the 8 cores (6250 dsts/core); each core
    owns all edges into its dst range (~200K edges), sorted by dst and grouped
    into 128-dst chunks, padded to 128-edge tiles (pad slots marked dstcmp=-1).
  - The host replicates node features along edges (the "replicated node
    features" sharding): per core it stages x[src[e]]^T and x[dst[e]]^T as two
    big sequential fp32 streams. This avoids per-edge DMA gathers entirely
    (SWDGE descriptor generation is ~8ns/row and would dominate).
  - On device, per 128-edge tile:
      k_e|v_e = xsT_tile.T @ [Wk^T | Wv^T]   (PE, per-edge projection)
      q_e     = xdT_tile.T @ Wq^T            (PE)
      scores  = head-wise dot(k_e, q_e)      (DVE mul + grouped reduce)
      p       = exp(scores)                  (ACT; no max-subtraction needed:
                                              |scores| < ~40 so fp32 exp is
                                              safe, and softmax is shift-free)
      pv      = v_e * p                      (GPSIMD)
      per-dst-chunk segmented sums of [pv | p] via one-hot matmul (PE) into a
      PSUM accumulator; finalize out = pv_sum / p_sum per chunk, DMA out.
  - Output: concat of per-core [6250,128] blocks -> [50000,1,128].
"""
import math
import numpy as np

import concourse.bass as bass
import concourse.tile as tile
from concourse import bacc, mybir
from concourse.bass_utils import run_bass_kernel_spmd

f32 = mybir.dt.float32

# problem shape (hardcoded per contract)
N = 50000
E = 1600000
D = 128
H = 4
NC = 8
NDST = N // NC          # 6250 dsts per core
CH = 128                # dsts per chunk
NCHUNK = (NDST + CH - 1) // CH   # 49
GT = 4                  # tiles per DVE/ACT batch group
TB = 128                # edges per tile


def _schedule(src, dst):
    """Sort/pad edges per core; return common tile counts + per-core slot arrays."""
    core = dst // NDST
    dstl = dst % NDST
    per_core = []
    counts = np.zeros((NC, NCHUNK), np.int64)
    for c in range(NC):
        sel = np.nonzero(core == c)[0]
        order = np.argsort(dstl[sel], kind="stable")
        e = sel[order]
        chunk = dstl[e] // CH
        counts[c] = np.bincount(chunk, minlength=NCHUNK)
        per_core.append((e, dstl[e]))
    T = np.maximum(1, np.ceil(counts.max(axis=0) / TB).astype(np.int64))  # [NCHUNK]
    NT = int(T.sum())
    ETOT = NT * TB
    tile_base = np.concatenate([[0], np.cumsum(T)])  # chunk -> first tile
    slots_src = np.zeros((NC, ETOT), np.int64)
    slots_dst = np.zeros((NC, ETOT), np.int64)
    dcmp = np.full((NC, ETOT), -1.0, np.float32)
    for c in range(NC):
        e, dl = per_core[c]
        pos = 0
        for j in range(NCHUNK):
            n = counts[c, j]
            base = int(tile_base[j]) * TB
            slots_src[c, base:base + n] = src[e[pos:pos + n]]
            slots_dst[c, base:base + n] = dst[e[pos:pos + n]]
            dcmp[c, base:base + n] = (dl[pos:pos + n] - j * CH).astype(np.float32)
            pos += n
    return T, tile_base, slots_src, slots_dst, dcmp


def _build(T, has_bias):
    """Emit the SPMD Bass program for common schedule T (tiles per chunk)."""
    T = [int(t) for t in T]
    NT = sum(T)
    ETOT = NT * TB
    nc = bacc.Bacc("TRN2", target_bir_lowering=False, debug=False, num_devices=NC)
    xsT = nc.dram_tensor("xsT", [128, ETOT], f32, kind="ExternalInput").ap()
    xdT = nc.dram_tensor("xdT", [128, ETOT], f32, kind="ExternalInput").ap()
    dcmp_d = nc.dram_tensor("dcmp", [128, NT, 1], f32, kind="ExternalInput").ap()
    wk_d = nc.dram_tensor("Wk", [128, 128], f32, kind="ExternalInput").ap()
    wq_d = nc.dram_tensor("Wq", [128, 128], f32, kind="ExternalInput").ap()
    wv_d = nc.dram_tensor("Wv", [128, 128], f32, kind="ExternalInput").ap()
    ident_d = nc.dram_tensor("ident", [128, 128], f32, kind="ExternalInput").ap()
    iota_d = nc.dram_tensor("iota4", [128, GT * 128], f32, kind="ExternalInput").ap()
    if has_bias:
        bkv_d = nc.dram_tensor("bkv", [1, 256], f32, kind="ExternalInput").ap()
        bq_d = nc.dram_tensor("bq", [1, 128], f32, kind="ExternalInput").ap()
        ones_d = nc.dram_tensor("ones", [1, 128], f32, kind="ExternalInput").ap()
    out_d = nc.dram_tensor("out", [NDST, 128], f32, kind="ExternalOutput").ap()

    with tile.TileContext(nc) as tc:
        with (
            tc.tile_pool(name="const", bufs=1) as cpool,
            tc.tile_pool(name="xs", bufs=3) as xspool,
            tc.tile_pool(name="xd", bufs=3) as xdpool,
            tc.tile_pool(name="kvsb", bufs=3) as kvsbpool,
            tc.tile_pool(name="work", bufs=2) as wpool,
            tc.tile_pool(name="dc", bufs=2) as dcpool,
            tc.tile_pool(name="kvps", bufs=2, space="PSUM") as kvpspool,
            tc.tile_pool(name="qps", bufs=2, space="PSUM") as qpspool,
            tc.tile_pool(name="aggps", bufs=2, space="PSUM") as aggpool,
        ):
            # ---- setup: load consts, transpose Wk/Wq/Wv on PE ----
            wk_sb = cpool.tile([128, 128], f32, tag="wk")
            wq_sb = cpool.tile([128, 128], f32, tag="wq")
            wv_sb = cpool.tile([128, 128], f32, tag="wv")
            ident = cpool.tile([128, 128], f32, tag="ident")
            iota4 = cpool.tile([128, GT * 128], f32, tag="iota4")
            nc.scalar.dma_start(out=wk_sb[:], in_=wk_d[:])
            nc.scalar.dma_start(out=wq_sb[:], in_=wq_d[:])
            nc.scalar.dma_start(out=wv_sb[:], in_=wv_d[:])
            nc.scalar.dma_start(out=ident[:], in_=ident_d[:])
            nc.scalar.dma_start(out=iota4[:], in_=iota_d[:])
            wkvT = cpool.tile([128, 256], f32, tag="wkvT")   # [Wk^T | Wv^T]
            wqT = cpool.tile([128, 128], f32, tag="wqT")
            if has_bias:
                bkv_sb = cpool.tile([1, 256], f32, tag="bkv")
                bq_sb = cpool.tile([1, 128], f32, tag="bq")
                ones_sb = cpool.tile([1, 128], f32, tag="ones")
                nc.scalar.dma_start(out=bkv_sb[:], in_=bkv_d[:])
                nc.scalar.dma_start(out=bq_sb[:], in_=bq_d[:])
                nc.scalar.dma_start(out=ones_sb[:], in_=ones_d[:])
            for w_sb, dst_slice in ((wk_sb, wkvT[:, 0:128]),
                                    (wv_sb, wkvT[:, 128:256]),
                                    (wq_sb, wqT[:])):
                tp = aggpool.tile([128, 132, 1], f32, tag="agg")
                nc.tensor.transpose(out=tp[:, 0:128, 0], in_=w_sb[:], identity=ident[:])
                nc.scalar.copy(out=dst_slice, in_=tp[:, 0:128, 0])

            # ---- edge phase ----
            for j in range(NCHUNK):
                tbase = sum(T[:j])
                tj = T[j]
                agg = aggpool.tile([128, 132, 1], f32, tag="agg")
                dc_t = dcpool.tile([128, max(T), 1], f32, tag="dc")
                nc.scalar.dma_start(out=dc_t[:, :tj, :],
                                    in_=dcmp_d[:, tbase:tbase + tj, :])
                done = 0
                while done < tj:
                    r = min(GT, tj - done)
                    g0 = (tbase + done) * TB   # first edge column of group
                    xs_t = xspool.tile([128, GT * TB], f32, tag="xs")
                    xd_t = xdpool.tile([128, GT * TB], f32, tag="xd")
                    nc.scalar.dma_start(out=xs_t[:, : r * TB],
                                        in_=xsT[:, g0:g0 + r * TB])
                    nc.scalar.dma_start(out=xd_t[:, : r * TB],
                                        in_=xdT[:, g0:g0 + r * TB])
                    kv_ps = kvpspool.tile([128, GT, 256], f32, tag="kvps")
                    q_ps = qpspool.tile([128, GT * 128], f32, tag="qps")
                    for i in range(r):
                        sl = slice(i * 128, (i + 1) * 128)
                        nc.tensor.matmul(out=kv_ps[:, i, :], lhsT=xs_t[:, sl],
                                         rhs=wkvT[:], start=True, stop=not has_bias,
                                         skip_group_check=True)
                        if has_bias:
                            nc.tensor.matmul(out=kv_ps[:, i, :], lhsT=ones_sb[0:1, :],
                                             rhs=bkv_sb[0:1, :], start=False,
                                             stop=True, skip_group_check=True)
                        nc.tensor.matmul(out=q_ps[:, sl], lhsT=xd_t[:, sl],
                                         rhs=wqT[:], start=True, stop=not has_bias,
                                         skip_group_check=True)
                        if has_bias:
                            nc.tensor.matmul(out=q_ps[:, sl], lhsT=ones_sb[0:1, :],
                                             rhs=bq_sb[0:1, :], start=False,
                                             stop=True, skip_group_check=True)
                    kv_sb = kvsbpool.tile([128, GT, 256], f32, tag="kvsb")
                    nc.scalar.copy(out=kv_sb[:, :r, :], in_=kv_ps[:, :r, :])
                    sel_e = wpool.tile([128, GT, 128], f32, tag="sel")
                    nc.vector.tensor_tensor(
                        out=sel_e[:, :r, :],
                        in0=dc_t[:, done:done + r, :].to_broadcast([128, r, 128]),
                        in1=iota4[:, : r * 128].rearrange("p (r c) -> p r c", r=r),
                        op=mybir.AluOpType.is_equal,
                    )
                    prod = wpool.tile([128, GT, H, 32], f32, tag="prod")
                    nc.vector.tensor_tensor(
                        out=prod[:, :r],
                        in0=kv_sb[:, :r, 0:128].rearrange("p r (h c) -> p r h c", h=H),
                        in1=q_ps[:, : r * 128].rearrange("p (r h c) -> p r h c",
                                                         r=r, h=H),
                        op=mybir.AluOpType.mult,
                    )
                    scores = wpool.tile([128, GT, H], f32, tag="scores")
                    nc.vector.tensor_reduce(out=scores[:, :r, :], in_=prod[:, :r],
                                            axis=mybir.AxisListType.X,
                                            op=mybir.AluOpType.add)
                    pvp = wpool.tile([128, GT, 132, 1], f32, tag="pvp")
                    nc.scalar.activation(out=pvp[:, :r, 128:132, 0],
                                         in_=scores[:, :r, :],
                                         func=mybir.ActivationFunctionType.Exp)
                    nc.gpsimd.tensor_tensor(
                        out=pvp[:, :r, 0:128, 0].rearrange("p r (h c) -> p r h c",
                                                           h=H),
                        in0=kv_sb[:, :r, 128:256].rearrange("p r (h c) -> p r h c",
                                                            h=H),
                        in1=pvp[:, :r, 128:132, :].to_broadcast([128, r, H, 32]),
                        op=mybir.AluOpType.mult,
                    )
                    for i in range(r):
                        nc.tensor.matmul(out=agg[:, :, 0], lhsT=sel_e[:, i, :],
                                         rhs=pvp[:, i, :, 0],
                                         start=(done + i == 0),
                                         stop=(done + i == tj - 1),
                                         skip_group_check=True)
                    done += r
                # finalize chunk
                den = wpool.tile([128, H, 1], f32, tag="den")
                nc.vector.tensor_scalar_max(den[:], agg[:, 128:132, :], 1e-30)
                rec = wpool.tile([128, H, 1], f32, tag="rec")
                nc.vector.reciprocal(rec[:], den[:])
                outn = wpool.tile([128, H, 32], f32, tag="outn")
                nc.vector.tensor_tensor(
                    out=outn[:],
                    in0=agg[:, 0:128, 0].rearrange("p (h c) -> p h c", h=H),
                    in1=rec[:].to_broadcast([128, H, 32]),
                    op=mybir.AluOpType.mult,
                )
                rows = min(CH, NDST - j * CH)
                nc.scalar.dma_start(
                    out=out_d[j * CH: j * CH + rows, :],
                    in_=outn[:rows].rearrange("p h c -> p (h c)"),
                )
    nc.compile()
    return nc


def kernel(**inputs):
    x = np.ascontiguousarray(np.asarray(inputs["x"], np.float32))
    Wk = np.ascontiguousarray(np.asarray(inputs["Wk"], np.float32))
    Wq = np.ascontiguousarray(np.asarray(inputs["Wq"], np.float32))
    Wv = np.ascontiguousarray(np.asarray(inputs["Wv"], np.float32))
    bk = np.asarray(inputs["bk"], np.float32)
    bq = np.asarray(inputs["bq"], np.float32)
    bv = np.asarray(inputs["bv"], np.float32)
    src = np.asarray(inputs["src"]).astype(np.int64)
    dst = np.asarray(inputs["dst"]).astype(np.int64)

    has_bias = bool(bk.any() or bq.any() or bv.any())
    T, tile_base, slots_src, slots_dst, dcmp = _schedule(src, dst)
    nc = _build(T, has_bias)

    ident = np.eye(128, dtype=np.float32)
    iota4 = np.tile(np.arange(128, dtype=np.float32), (128, GT))
    in_maps = []
    for c in range(NC):
        xs = np.ascontiguousarray(x[slots_src[c]].T)          # [128, ETOT]
        xd = np.ascontiguousarray(x[slots_dst[c]].T)
        m = {
            "xsT": xs,
            "xdT": xd,
            "dcmp": np.ascontiguousarray(dcmp[c].reshape(-1, TB).T)[:, :, None],
            "Wk": Wk, "Wq": Wq, "Wv": Wv,
            "ident": ident, "iota4": iota4,
        }
        if has_bias:
            m["bkv"] = np.concatenate([bk, bv]).reshape(1, 256).astype(np.float32)
            m["bq"] = bq.reshape(1, 128).astype(np.float32)
            m["ones"] = np.ones((1, 128), np.float32)
        in_maps.append(m)

    import os
    trace_dir = os.environ.get("BASS_GAT_TRACE")
    kw = {}
    if trace_dir:
        os.makedirs(trace_dir, exist_ok=True)
        kw = dict(trace=True, tmpdir=trace_dir)
    res = run_bass_kernel_spmd(nc, in_maps, core_ids=list(range(NC)), **kw)
    if trace_dir and res.exec_time_ns is not None:
        print(f"HW exec time: {res.exec_time_ns} ns")
    out = np.concatenate([res.results[c]["out"] for c in range(NC)], axis=0)
    return out.reshape(N, 1, D).astype(np.float32)


if __name__ == "__main__":
    # tiny self-check with random data against a numpy reference
    rng = np.random.default_rng(0)
    ins = {
        "x": rng.standard_normal((N, D), np.float32),
        "Wk": (rng.standard_normal((D, D)) / math.sqrt(D)).astype(np.float32),
        "bk": np.zeros(D, np.float32),
        "Wq": (rng.standard_normal((D, D)) / math.sqrt(D)).astype(np.float32),
        "bq": np.zeros(D, np.float32),
        "Wv": (rng.standard_normal((D, D)) / math.sqrt(D)).astype(np.float32),
        "bv": np.zeros(D, np.float32),
        "src": rng.integers(0, N, E).astype(np.int32),
        "dst": rng.integers(0, N, E).astype(np.int32),
    }
    out = kernel(**ins)
    print("out", out.shape, out.dtype, np.abs(out).max())


# revision 7
# speedup vs baseline: 1.4598x; 1.4598x over previous
"""GAT layer (nn_GATLayer_32719060861314) as a Bass/Tile SPMD kernel on 8 trn2 cores.

Strategy (edge-sharded, dst-partitioned, gather-free):
  - Node dsts are range-partitioned over the 8 cores (6250 dsts/core); each core
    owns all edges into its dst range (~200K edges), sorted by dst and grouped
    into 128-dst chunks, padded to 128-edge tiles (pad slots marked dstcmp=-1).
  - The host replicates node features along edges (the "replicated node
    features" sharding): per core it stages x[src[e]]^T and x[dst[e]]^T as two
    big sequential fp32 streams. This avoids per-edge DMA gathers entirely
    (SWDGE descriptor generation is ~8ns/row and would dominate).
  - On device, per 128-edge tile:
      k_e|v_e = xsT_tile.T @ [Wk^T | Wv^T]   (PE, per-edge projection)
      q_e     = xdT_tile.T @ Wq^T            (PE)
      scores  = head-wise dot(k_e, q_e)      (DVE mul + grouped reduce)
      p       = exp(scores)                  (ACT; no max-subtraction needed:
                                              |scores| < ~40 so fp32 exp is
                                              safe, and softmax is shift-free)
      pv      = v_e * p                      (GPSIMD)
      per-dst-chunk segmented sums of [pv | p] via one-hot matmul (PE) into a
      PSUM accumulator; finalize out = pv_sum / p_sum per chunk, DMA out.
  - Output: concat of per-core [6250,128] blocks -> [50000,1,128].
"""
import math
import numpy as np

import concourse.bass as bass
import concourse.tile as tile
from concourse import bacc, mybir
from concourse.bass_utils import run_bass_kernel_spmd

f32 = mybir.dt.float32
f32r = mybir.dt.float32r
bf16 = mybir.dt.bfloat16

# problem shape (hardcoded per contract)
N = 50000
E = 1600000
D = 128
H = 4
NC = 8
NDST = N // NC          # 6250 dsts per core
CH = 128                # dsts per chunk
NCHUNK = (NDST + CH - 1) // CH   # 49
GT = 4                  # tiles per DVE/ACT batch group
TB = 128                # edges per tile


def _schedule(src, dst):
    """Sort/pad edges per core; return common tile counts + per-core slot arrays."""
    core = dst // NDST
    dstl = dst % NDST
    per_core = []
    counts = np.zeros((NC, NCHUNK), np.int64)
    for c in range(NC):
        sel = np.nonzero(core == c)[0]
        order = np.argsort(dstl[sel], kind="stable")
        e = sel[order]
        chunk = dstl[e] // CH
        counts[c] = np.bincount(chunk, minlength=NCHUNK)
        per_core.append((e, dstl[e]))
    T = np.maximum(1, np.ceil(counts.max(axis=0) / TB).astype(np.int64))  # [NCHUNK]
    NT = int(T.sum())
    ETOT = NT * TB
    tile_base = np.concatenate([[0], np.cumsum(T)])  # chunk -> first tile
    slots_src = np.zeros((NC, ETOT), np.int64)
    slots_dst = np.zeros((NC, ETOT), np.int64)
    dcmp = np.full((NC, ETOT), -1.0, np.float32)
    for c in range(NC):
        e, dl = per_core[c]
        pos = 0
        for j in range(NCHUNK):
            n = counts[c, j]
            base = int(tile_base[j]) * TB
            slots_src[c, base:base + n] = src[e[pos:pos + n]]
            slots_dst[c, base:base + n] = dst[e[pos:pos + n]]
            dcmp[c, base:base + n] = (dl[pos:pos + n] - j * CH).astype(np.float32)
            pos += n
    return T, tile_base, slots_src, slots_dst, dcmp


def _build(T, has_bias):
    """Emit the SPMD Bass program for common schedule T (tiles per chunk)."""
    T = [int(t) for t in T]
    NT = sum(T)
    ETOT = NT * TB
    nc = bacc.Bacc("TRN2", target_bir_lowering=False, debug=False, num_devices=NC)
    xsT = nc.dram_tensor("xsT", [128, ETOT], f32r, kind="ExternalInput").ap()
    xdT = nc.dram_tensor("xdT", [128, ETOT], f32r, kind="ExternalInput").ap()
    dcmp_d = nc.dram_tensor("dcmp", [128, NT, 1], f32, kind="ExternalInput").ap()
    wk_d = nc.dram_tensor("Wk", [128, 128], f32, kind="ExternalInput").ap()
    wq_d = nc.dram_tensor("Wq", [128, 128], f32, kind="ExternalInput").ap()
    wv_d = nc.dram_tensor("Wv", [128, 128], f32, kind="ExternalInput").ap()
    ident_d = nc.dram_tensor("ident", [128, 128], f32, kind="ExternalInput").ap()
    iota_d = nc.dram_tensor("iota4", [128, GT * 128], f32, kind="ExternalInput").ap()
    if has_bias:
        bkv_d = nc.dram_tensor("bkv", [1, 256], f32, kind="ExternalInput").ap()
        bq_d = nc.dram_tensor("bq", [1, 128], f32, kind="ExternalInput").ap()
        ones_d = nc.dram_tensor("ones", [1, 128], f32, kind="ExternalInput").ap()
    out_d = nc.dram_tensor("out", [NDST, 128], f32, kind="ExternalOutput").ap()

    with tile.TileContext(nc) as tc:
        with (
            tc.tile_pool(name="const", bufs=1) as cpool,
            tc.tile_pool(name="xs", bufs=3) as xspool,
            tc.tile_pool(name="xd", bufs=3) as xdpool,
            tc.tile_pool(name="kvsb", bufs=3) as kvsbpool,
            tc.tile_pool(name="work", bufs=2) as wpool,
            tc.tile_pool(name="dc", bufs=2) as dcpool,
            tc.tile_pool(name="kvps", bufs=1, space="PSUM") as kvpspool,
            tc.tile_pool(name="qps", bufs=2, space="PSUM") as qpspool,
            tc.tile_pool(name="aggps", bufs=2, space="PSUM") as aggpool,
        ):
            # ---- setup: load consts, transpose Wk/Wq/Wv on PE ----
            wk_sb = cpool.tile([128, 128], f32, tag="wk")
            wq_sb = cpool.tile([128, 128], f32, tag="wq")
            wv_sb = cpool.tile([128, 128], f32, tag="wv")
            ident = cpool.tile([128, 128], f32, tag="ident")
            iota4 = cpool.tile([128, GT * 128], f32, tag="iota4")
            nc.scalar.dma_start(out=wk_sb[:], in_=wk_d[:])
            nc.scalar.dma_start(out=wq_sb[:], in_=wq_d[:])
            nc.scalar.dma_start(out=wv_sb[:], in_=wv_d[:])
            nc.scalar.dma_start(out=ident[:], in_=ident_d[:])
            nc.scalar.dma_start(out=iota4[:], in_=iota_d[:])
            wkvT = cpool.tile([128, 256], f32r, tag="wkvT")  # [Wk^T | Wv^T] (f32r)
            wqT = cpool.tile([128, 256], f32r, tag="wqT")
            if has_bias:
                bkv_sb = cpool.tile([1, 256], f32, tag="bkv")
                bq_sb = cpool.tile([1, 128], f32, tag="bq")
                ones_sb = cpool.tile([1, 128], f32, tag="ones")
                nc.scalar.dma_start(out=bkv_sb[:], in_=bkv_d[:])
                nc.scalar.dma_start(out=bq_sb[:], in_=bq_d[:])
                nc.scalar.dma_start(out=ones_sb[:], in_=ones_d[:])
            for w_sb, dst_slices in ((wk_sb, [wkvT[:, 0:128]]),
                                     (wv_sb, [wkvT[:, 128:256]]),
                                     (wq_sb, [wqT[:, 0:128], wqT[:, 128:256]])):
                tp = aggpool.tile([128, 256, 1], f32, tag="agg")
                nc.tensor.transpose(out=tp[:, 0:128, 0], in_=w_sb[:], identity=ident[:])
                for ds in dst_slices:
                    nc.scalar.copy(out=ds, in_=tp[:, 0:128, 0])

            # ---- edge phase ----
            for j in range(NCHUNK):
                tbase = sum(T[:j])
                tj = T[j]
                agg = aggpool.tile([128, 256, 1], f32, tag="agg")
                dc_t = dcpool.tile([128, max(T), 1], f32, tag="dc")
                nc.sync.dma_start(out=dc_t[:, :tj, :],
                                    in_=dcmp_d[:, tbase:tbase + tj, :])
                done = 0
                while done < tj:
                    r = min(GT, tj - done)
                    g0 = (tbase + done) * TB   # first edge column of group
                    xs_t = xspool.tile([128, GT * TB], f32r, tag="xs")
                    xd_t = xdpool.tile([128, GT * TB], f32r, tag="xd")
                    nc.sync.dma_start(out=xs_t[:, : r * TB],
                                        in_=xsT[:, g0:g0 + r * TB])
                    nc.sync.dma_start(out=xd_t[:, : r * TB],
                                        in_=xdT[:, g0:g0 + r * TB])
                    kv_ps = kvpspool.tile([128, GT, 256], f32, tag="kvps")
                    q_ps = qpspool.tile([128, GT, 256], f32, tag="qps")
                    for i in range(r):
                        sl = slice(i * 128, (i + 1) * 128)
                        nc.tensor.matmul(out=kv_ps[:, i, :], lhsT=xs_t[:, sl],
                                         rhs=wkvT[:], start=True, stop=not has_bias,
                                         skip_group_check=True)
                        if has_bias:
                            nc.tensor.matmul(out=kv_ps[:, i, :], lhsT=ones_sb[0:1, :],
                                             rhs=bkv_sb[0:1, :], start=False,
                                             stop=True, skip_group_check=True)
                        nc.tensor.matmul(out=q_ps[:, i, :], lhsT=xd_t[:, sl],
                                         rhs=wqT[:], start=True, stop=not has_bias,
                                         skip_group_check=True)
                        if has_bias:
                            nc.tensor.matmul(out=q_ps[:, i, 0:128], lhsT=ones_sb[0:1, :],
                                             rhs=bq_sb[0:1, :], start=False,
                                             stop=True, skip_group_check=True)
                    k_sb = kvsbpool.tile([128, GT, 128], f32, tag="ksb")
                    nc.scalar.copy(out=k_sb[:, :r, :], in_=kv_ps[:, :r, 0:128])
                    v_sb = kvsbpool.tile([128, GT, 128], f32r, tag="vsb")
                    nc.scalar.copy(out=v_sb[:, :r, :], in_=kv_ps[:, :r, 128:256])
                    sel_e = wpool.tile([128, GT, 128], f32r, tag="sel")
                    nc.vector.tensor_tensor(
                        out=sel_e[:, :r, :],
                        in0=dc_t[:, done:done + r, :].to_broadcast([128, r, 128]),
                        in1=iota4[:, : r * 128].rearrange("p (r c) -> p r c", r=r),
                        op=mybir.AluOpType.is_equal,
                    )
                    prod = wpool.tile([128, GT, H, 32], f32, tag="prod")
                    nc.vector.tensor_tensor(
                        out=prod[:, :r],
                        in0=k_sb[:, :r, :].rearrange("p r (h c) -> p r h c", h=H),
                        in1=q_ps[:, :r, 0:128].rearrange("p r (h c) -> p r h c", h=H),
                        op=mybir.AluOpType.mult,
                    )
                    scores = wpool.tile([128, GT, H], f32, tag="scores")
                    nc.vector.tensor_reduce(out=scores[:, :r, :], in_=prod[:, :r],
                                            axis=mybir.AxisListType.X,
                                            op=mybir.AluOpType.add)
                    pvp = wpool.tile([128, GT, 256, 1], f32r, tag="pvp")
                    nc.scalar.activation(out=pvp[:, :r, 128:132, 0],
                                         in_=scores[:, :r, :],
                                         func=mybir.ActivationFunctionType.Exp)
                    nc.gpsimd.tensor_tensor(
                        out=pvp[:, :r, 0:128, 0].rearrange("p r (h c) -> p r h c",
                                                           h=H),
                        in0=v_sb[:, :r, :].rearrange("p r (h c) -> p r h c", h=H),
                        in1=pvp[:, :r, 128:132, :].to_broadcast([128, r, H, 32]),
                        op=mybir.AluOpType.mult,
                    )
                    for i in range(r):
                        nc.tensor.matmul(out=agg[:, :, 0], lhsT=sel_e[:, i, :],
                                         rhs=pvp[:, i, :, 0],
                                         start=(done + i == 0),
                                         stop=(done + i == tj - 1),
                                         skip_group_check=True)
                    done += r
                # finalize chunk
                den = wpool.tile([128, H, 1], f32, tag="den")
                nc.vector.tensor_scalar_max(den[:], agg[:, 128:132, :], 1e-30)
                rec = wpool.tile([128, H, 1], f32, tag="rec")
                nc.vector.reciprocal(rec[:], den[:])
                outn = wpool.tile([128, H, 32], f32, tag="outn")
                nc.vector.tensor_tensor(
                    out=outn[:],
                    in0=agg[:, 0:128, 0].rearrange("p (h c) -> p h c", h=H),
                    in1=rec[:].to_broadcast([128, H, 32]),
                    op=mybir.AluOpType.mult,
                )
                rows = min(CH, NDST - j * CH)
                nc.sync.dma_start(
                    out=out_d[j * CH: j * CH + rows, :],
                    in_=outn[:rows].rearrange("p h c -> p (h c)"),
                )
    nc.compile()
    return nc


def kernel(**inputs):
    x = np.ascontiguousarray(np.asarray(inputs["x"], np.float32))
    Wk = np.ascontiguousarray(np.asarray(inputs["Wk"], np.float32))
    Wq = np.ascontiguousarray(np.asarray(inputs["Wq"], np.float32))
    Wv = np.ascontiguousarray(np.asarray(inputs["Wv"], np.float32))
    bk = np.asarray(inputs["bk"], np.float32)
    bq = np.asarray(inputs["bq"], np.float32)
    bv = np.asarray(inputs["bv"], np.float32)
    src = np.asarray(inputs["src"]).astype(np.int64)
    dst = np.asarray(inputs["dst"]).astype(np.int64)

    has_bias = bool(bk.any() or bq.any() or bv.any())
    T, tile_base, slots_src, slots_dst, dcmp = _schedule(src, dst)
    nc = _build(T, has_bias)

    ident = np.eye(128, dtype=np.float32)
    iota4 = np.tile(np.arange(128, dtype=np.float32), (128, GT))
    in_maps = []
    for c in range(NC):
        xs = np.ascontiguousarray(x[slots_src[c]].T)          # [128, ETOT]
        xd = np.ascontiguousarray(x[slots_dst[c]].T)
        m = {
            "xsT": xs,
            "xdT": xd,
            "dcmp": np.ascontiguousarray(dcmp[c].reshape(-1, TB).T)[:, :, None],
            "Wk": Wk, "Wq": Wq, "Wv": Wv,
            "ident": ident, "iota4": iota4,
        }
        if has_bias:
            m["bkv"] = np.concatenate([bk, bv]).reshape(1, 256).astype(np.float32)
            m["bq"] = bq.reshape(1, 128).astype(np.float32)
            m["ones"] = np.ones((1, 128), np.float32)
        in_maps.append(m)

    import os
    trace_dir = os.environ.get("BASS_GAT_TRACE")
    kw = {}
    if trace_dir:
        os.makedirs(trace_dir, exist_ok=True)
        kw = dict(trace=True, tmpdir=trace_dir)
    res = run_bass_kernel_spmd(nc, in_maps, core_ids=list(range(NC)), **kw)
    if trace_dir and res.exec_time_ns is not None:
        print(f"HW exec time: {res.exec_time_ns} ns")
    out = np.concatenate([res.results[c]["out"] for c in range(NC)], axis=0)
    return out.reshape(N, 1, D).astype(np.float32)


if __name__ == "__main__":
    # tiny self-check with random data against a numpy reference
    rng = np.random.default_rng(0)
    ins = {
        "x": rng.standard_normal((N, D), np.float32),
        "Wk": (rng.standard_normal((D, D)) / math.sqrt(D)).astype(np.float32),
        "bk": np.zeros(D, np.float32),
        "Wq": (rng.standard_normal((D, D)) / math.sqrt(D)).astype(np.float32),
        "bq": np.zeros(D, np.float32),
        "Wv": (rng.standard_normal((D, D)) / math.sqrt(D)).astype(np.float32),
        "bv": np.zeros(D, np.float32),
        "src": rng.integers(0, N, E).astype(np.int32),
        "dst": rng.integers(0, N, E).astype(np.int32),
    }
    out = kernel(**ins)
    print("out", out.shape, out.dtype, np.abs(out).max())


# revision 9
# speedup vs baseline: 1.4616x; 1.0012x over previous
"""GAT layer (nn_GATLayer_32719060861314) as a Bass/Tile SPMD kernel on 8 trn2 cores.

Strategy (edge-sharded, dst-partitioned, gather-free):
  - Node dsts are range-partitioned over the 8 cores (6250 dsts/core); each core
    owns all edges into its dst range (~200K edges), sorted by dst and grouped
    into 128-dst chunks, padded to 128-edge tiles (pad slots marked dstcmp=-1).
  - The host replicates node features along edges (the "replicated node
    features" sharding): per core it stages x[src[e]]^T and x[dst[e]]^T as two
    big sequential fp32 streams. This avoids per-edge DMA gathers entirely
    (SWDGE descriptor generation is ~8ns/row and would dominate).
  - On device, per 128-edge tile:
      k_e|v_e = xsT_tile.T @ [Wk^T | Wv^T]   (PE, per-edge projection)
      q_e     = xdT_tile.T @ Wq^T            (PE)
      scores  = head-wise dot(k_e, q_e)      (DVE mul + grouped reduce)
      p       = exp(scores)                  (ACT; no max-subtraction needed:
                                              |scores| < ~40 so fp32 exp is
                                              safe, and softmax is shift-free)
      pv      = v_e * p                      (GPSIMD)
      per-dst-chunk segmented sums of [pv | p] via one-hot matmul (PE) into a
      PSUM accumulator; finalize out = pv_sum / p_sum per chunk, DMA out.
  - Output: concat of per-core [6250,128] blocks -> [50000,1,128].
"""
import math
import numpy as np

import concourse.bass as bass
import concourse.tile as tile
from concourse import bacc, mybir
from concourse.bass_utils import run_bass_kernel_spmd

f32 = mybir.dt.float32
f32r = mybir.dt.float32r
bf16 = mybir.dt.bfloat16

# problem shape (hardcoded per contract)
N = 50000
E = 1600000
D = 128
H = 4
NC = 8
NDST = N // NC          # 6250 dsts per core
CH = 128                # dsts per chunk
NCHUNK = (NDST + CH - 1) // CH   # 49
GT = 4                  # tiles per DVE/ACT batch group
TB = 128                # edges per tile


def _schedule(src, dst):
    """Sort/pad edges per core; return common tile counts + per-core slot arrays."""
    core = dst // NDST
    dstl = dst % NDST
    per_core = []
    counts = np.zeros((NC, NCHUNK), np.int64)
    for c in range(NC):
        sel = np.nonzero(core == c)[0]
        order = np.argsort(dstl[sel], kind="stable")
        e = sel[order]
        chunk = dstl[e] // CH
        counts[c] = np.bincount(chunk, minlength=NCHUNK)
        per_core.append((e, dstl[e]))
    T = np.maximum(1, np.ceil(counts.max(axis=0) / TB).astype(np.int64))  # [NCHUNK]
    NT = int(T.sum())
    ETOT = NT * TB
    tile_base = np.concatenate([[0], np.cumsum(T)])  # chunk -> first tile
    slots_src = np.zeros((NC, ETOT), np.int64)
    slots_dst = np.zeros((NC, ETOT), np.int64)
    dcmp = np.full((NC, ETOT), -1.0, np.float32)
    for c in range(NC):
        e, dl = per_core[c]
        pos = 0
        for j in range(NCHUNK):
            n = counts[c, j]
            base = int(tile_base[j]) * TB
            slots_src[c, base:base + n] = src[e[pos:pos + n]]
            slots_dst[c, base:base + n] = dst[e[pos:pos + n]]
            dcmp[c, base:base + n] = (dl[pos:pos + n] - j * CH).astype(np.float32)
            pos += n
    return T, tile_base, slots_src, slots_dst, dcmp


def _build(T, has_bias):
    """Emit the SPMD Bass program for common schedule T (tiles per chunk)."""
    T = [int(t) for t in T]
    NT = sum(T)
    ETOT = NT * TB
    nc = bacc.Bacc("TRN2", target_bir_lowering=False, debug=False, num_devices=NC)
    xsT = nc.dram_tensor("xsT", [128, ETOT], f32r, kind="ExternalInput").ap()
    xdT = nc.dram_tensor("xdT", [128, ETOT], f32r, kind="ExternalInput").ap()
    dcmp_d = nc.dram_tensor("dcmp", [128, NT, 1], f32, kind="ExternalInput").ap()
    wk_d = nc.dram_tensor("Wk", [128, 128], f32, kind="ExternalInput").ap()
    wq_d = nc.dram_tensor("Wq", [128, 128], f32, kind="ExternalInput").ap()
    wv_d = nc.dram_tensor("Wv", [128, 128], f32, kind="ExternalInput").ap()
    ident_d = nc.dram_tensor("ident", [128, 128], f32, kind="ExternalInput").ap()
    iota_d = nc.dram_tensor("iota4", [128, GT * 128], f32, kind="ExternalInput").ap()
    if has_bias:
        bkv_d = nc.dram_tensor("bkv", [1, 256], f32, kind="ExternalInput").ap()
        bq_d = nc.dram_tensor("bq", [1, 128], f32, kind="ExternalInput").ap()
        ones_d = nc.dram_tensor("ones", [1, 128], f32, kind="ExternalInput").ap()
    out_d = nc.dram_tensor("out", [NDST, 128], f32, kind="ExternalOutput").ap()

    with tile.TileContext(nc) as tc:
        with (
            tc.tile_pool(name="const", bufs=1) as cpool,
            tc.tile_pool(name="xs", bufs=3) as xspool,
            tc.tile_pool(name="xd", bufs=3) as xdpool,
            tc.tile_pool(name="kvsb", bufs=3) as kvsbpool,
            tc.tile_pool(name="work", bufs=2) as wpool,
            tc.tile_pool(name="dc", bufs=2) as dcpool,
            tc.tile_pool(name="kvps", bufs=2, space="PSUM") as kvpspool,
            tc.tile_pool(name="qps", bufs=1, space="PSUM") as qpspool,
            tc.tile_pool(name="aggps", bufs=2, space="PSUM") as aggpool,
        ):
            # ---- setup: load consts, transpose Wk/Wq/Wv on PE ----
            wk_sb = cpool.tile([128, 128], f32, tag="wk")
            wq_sb = cpool.tile([128, 128], f32, tag="wq")
            wv_sb = cpool.tile([128, 128], f32, tag="wv")
            ident = cpool.tile([128, 128], f32, tag="ident")
            iota4 = cpool.tile([128, GT * 128], f32, tag="iota4")
            nc.scalar.dma_start(out=wk_sb[:], in_=wk_d[:])
            nc.scalar.dma_start(out=wq_sb[:], in_=wq_d[:])
            nc.scalar.dma_start(out=wv_sb[:], in_=wv_d[:])
            nc.scalar.dma_start(out=ident[:], in_=ident_d[:])
            nc.scalar.dma_start(out=iota4[:], in_=iota_d[:])
            wkvT = cpool.tile([128, 256], f32r, tag="wkvT")  # [Wk^T | Wv^T] (f32r)
            wqT = cpool.tile([128, 256], f32r, tag="wqT")
            if has_bias:
                bkv_sb = cpool.tile([1, 256], f32, tag="bkv")
                bq_sb = cpool.tile([1, 128], f32, tag="bq")
                ones_sb = cpool.tile([1, 128], f32, tag="ones")
                nc.scalar.dma_start(out=bkv_sb[:], in_=bkv_d[:])
                nc.scalar.dma_start(out=bq_sb[:], in_=bq_d[:])
                nc.scalar.dma_start(out=ones_sb[:], in_=ones_d[:])
            for w_sb, dst_slices in ((wk_sb, [wkvT[:, 0:128]]),
                                     (wv_sb, [wkvT[:, 128:256]]),
                                     (wq_sb, [wqT[:, 0:128], wqT[:, 128:256]])):
                tp = aggpool.tile([128, 256, 1], f32, tag="agg")
                nc.tensor.transpose(out=tp[:, 0:128, 0], in_=w_sb[:], identity=ident[:])
                for ds in dst_slices:
                    nc.scalar.copy(out=ds, in_=tp[:, 0:128, 0])

            # ---- edge phase ----
            for j in range(NCHUNK):
                tbase = sum(T[:j])
                tj = T[j]
                agg = aggpool.tile([128, 256, 1], f32, tag="agg")
                dc_t = dcpool.tile([128, max(T), 1], f32, tag="dc")
                nc.sync.dma_start(out=dc_t[:, :tj, :],
                                    in_=dcmp_d[:, tbase:tbase + tj, :])
                # stream xs/xd in 16-tile (1MB) blocks for DMA efficiency
                blocks = []
                t0b = 0
                BLK = 16
                while t0b < tj:
                    L = min(BLK, tj - t0b)
                    xs_b = xspool.tile([128, BLK * TB], f32r, tag="xs")
                    xd_b = xdpool.tile([128, BLK * TB], f32r, tag="xd")
                    c0 = (tbase + t0b) * TB
                    nc.sync.dma_start(out=xs_b[:, : L * TB],
                                      in_=xsT[:, c0:c0 + L * TB])
                    nc.sync.dma_start(out=xd_b[:, : L * TB],
                                      in_=xdT[:, c0:c0 + L * TB])
                    blocks.append((xs_b, xd_b))
                    t0b += L
                done = 0
                while done < tj:
                    r = min(GT, tj - done)
                    xs_bt, xd_bt = blocks[done // BLK]
                    loc = (done % BLK) * TB
                    xs_t = xs_bt[:, loc:loc + r * TB]
                    xd_t = xd_bt[:, loc:loc + r * TB]
                    kv_ps = kvpspool.tile([128, GT, 256], f32, tag="kvps")
                    q_ps = qpspool.tile([128, GT, 256], f32, tag="qps")
                    for i in range(r):
                        sl = slice(i * 128, (i + 1) * 128)
                        nc.tensor.matmul(out=kv_ps[:, i, :], lhsT=xs_t[:, sl],
                                         rhs=wkvT[:], start=True, stop=not has_bias,
                                         skip_group_check=True)
                        if has_bias:
                            nc.tensor.matmul(out=kv_ps[:, i, :], lhsT=ones_sb[0:1, :],
                                             rhs=bkv_sb[0:1, :], start=False,
                                             stop=True, skip_group_check=True)
                        nc.tensor.matmul(out=q_ps[:, i, :], lhsT=xd_t[:, sl],
                                         rhs=wqT[:], start=True, stop=not has_bias,
                                         skip_group_check=True)
                        if has_bias:
                            nc.tensor.matmul(out=q_ps[:, i, 0:128], lhsT=ones_sb[0:1, :],
                                             rhs=bq_sb[0:1, :], start=False,
                                             stop=True, skip_group_check=True)
                    k_sb = kvsbpool.tile([128, GT, 128], f32, tag="ksb")
                    nc.scalar.copy(out=k_sb[:, :r, :], in_=kv_ps[:, :r, 0:128])
                    v_sb = kvsbpool.tile([128, GT, 128], f32r, tag="vsb")
                    nc.scalar.copy(out=v_sb[:, :r, :], in_=kv_ps[:, :r, 128:256])
                    sel_e = wpool.tile([128, GT, 128], f32r, tag="sel")
                    nc.vector.tensor_tensor(
                        out=sel_e[:, :r, :],
                        in0=dc_t[:, done:done + r, :].to_broadcast([128, r, 128]),
                        in1=iota4[:, : r * 128].rearrange("p (r c) -> p r c", r=r),
                        op=mybir.AluOpType.is_equal,
                    )
                    prod = wpool.tile([128, GT, H, 32], f32, tag="prod")
                    nc.vector.tensor_tensor(
                        out=prod[:, :r],
                        in0=k_sb[:, :r, :].rearrange("p r (h c) -> p r h c", h=H),
                        in1=q_ps[:, :r, 0:128].rearrange("p r (h c) -> p r h c", h=H),
                        op=mybir.AluOpType.mult,
                    )
                    scores = wpool.tile([128, GT, H], f32, tag="scores")
                    nc.vector.tensor_reduce(out=scores[:, :r, :], in_=prod[:, :r],
                                            axis=mybir.AxisListType.X,
                                            op=mybir.AluOpType.add)
                    pvp = wpool.tile([128, GT, 256, 1], f32r, tag="pvp")
                    nc.scalar.activation(out=pvp[:, :r, 128:132, 0],
                                         in_=scores[:, :r, :],
                                         func=mybir.ActivationFunctionType.Exp)
                    nc.gpsimd.tensor_tensor(
                        out=pvp[:, :r, 0:128, 0].rearrange("p r (h c) -> p r h c",
                                                           h=H),
                        in0=v_sb[:, :r, :].rearrange("p r (h c) -> p r h c", h=H),
                        in1=pvp[:, :r, 128:132, :].to_broadcast([128, r, H, 32]),
                        op=mybir.AluOpType.mult,
                    )
                    for i in range(r):
                        nc.tensor.matmul(out=agg[:, :, 0], lhsT=sel_e[:, i, :],
                                         rhs=pvp[:, i, :, 0],
                                         start=(done + i == 0),
                                         stop=(done + i == tj - 1),
                                         skip_group_check=True)
                    done += r
                # finalize chunk
                den = wpool.tile([128, H, 1], f32, tag="den")
                nc.vector.tensor_scalar_max(den[:], agg[:, 128:132, :], 1e-30)
                rec = wpool.tile([128, H, 1], f32, tag="rec")
                nc.vector.reciprocal(rec[:], den[:])
                outn = wpool.tile([128, H, 32], f32, tag="outn")
                nc.vector.tensor_tensor(
                    out=outn[:],
                    in0=agg[:, 0:128, 0].rearrange("p (h c) -> p h c", h=H),
                    in1=rec[:].to_broadcast([128, H, 32]),
                    op=mybir.AluOpType.mult,
                )
                rows = min(CH, NDST - j * CH)
                nc.sync.dma_start(
                    out=out_d[j * CH: j * CH + rows, :],
                    in_=outn[:rows].rearrange("p h c -> p (h c)"),
                )
    nc.compile()
    return nc


def kernel(**inputs):
    x = np.ascontiguousarray(np.asarray(inputs["x"], np.float32))
    Wk = np.ascontiguousarray(np.asarray(inputs["Wk"], np.float32))
    Wq = np.ascontiguousarray(np.asarray(inputs["Wq"], np.float32))
    Wv = np.ascontiguousarray(np.asarray(inputs["Wv"], np.float32))
    bk = np.asarray(inputs["bk"], np.float32)
    bq = np.asarray(inputs["bq"], np.float32)
    bv = np.asarray(inputs["bv"], np.float32)
    src = np.asarray(inputs["src"]).astype(np.int64)
    dst = np.asarray(inputs["dst"]).astype(np.int64)

    has_bias = bool(bk.any() or bq.any() or bv.any())
    T, tile_base, slots_src, slots_dst, dcmp = _schedule(src, dst)
    nc = _build(T, has_bias)

    ident = np.eye(128, dtype=np.float32)
    iota4 = np.tile(np.arange(128, dtype=np.float32), (128, GT))
    in_maps = []
    for c in range(NC):
        xs = np.ascontiguousarray(x[slots_src[c]].T)          # [128, ETOT]
        xd = np.ascontiguousarray(x[slots_dst[c]].T)
        m = {
            "xsT": xs,
            "xdT": xd,
            "dcmp": np.ascontiguousarray(dcmp[c].reshape(-1, TB).T)[:, :, None],
            "Wk": Wk, "Wq": Wq, "Wv": Wv,
            "ident": ident, "iota4": iota4,
        }
        if has_bias:
            m["bkv"] = np.concatenate([bk, bv]).reshape(1, 256).astype(np.float32)
            m["bq"] = bq.reshape(1, 128).astype(np.float32)
            m["ones"] = np.ones((1, 128), np.float32)
        in_maps.append(m)

    import os
    trace_dir = os.environ.get("BASS_GAT_TRACE")
    kw = {}
    if trace_dir:
        os.makedirs(trace_dir, exist_ok=True)
        kw = dict(trace=True, tmpdir=trace_dir)
    res = run_bass_kernel_spmd(nc, in_maps, core_ids=list(range(NC)), **kw)
    if trace_dir and res.exec_time_ns is not None:
        print(f"HW exec time: {res.exec_time_ns} ns")
    out = np.concatenate([res.results[c]["out"] for c in range(NC)], axis=0)
    return out.reshape(N, 1, D).astype(np.float32)


if __name__ == "__main__":
    # tiny self-check with random data against a numpy reference
    rng = np.random.default_rng(0)
    ins = {
        "x": rng.standard_normal((N, D), np.float32),
        "Wk": (rng.standard_normal((D, D)) / math.sqrt(D)).astype(np.float32),
        "bk": np.zeros(D, np.float32),
        "Wq": (rng.standard_normal((D, D)) / math.sqrt(D)).astype(np.float32),
        "bq": np.zeros(D, np.float32),
        "Wv": (rng.standard_normal((D, D)) / math.sqrt(D)).astype(np.float32),
        "bv": np.zeros(D, np.float32),
        "src": rng.integers(0, N, E).astype(np.int32),
        "dst": rng.integers(0, N, E).astype(np.int32),
    }
    out = kernel(**ins)
    print("out", out.shape, out.dtype, np.abs(out).max())


# revision 10
# speedup vs baseline: 1.5000x; 1.0262x over previous
"""GAT layer (nn_GATLayer_32719060861314) as a Bass/Tile SPMD kernel on 8 trn2 cores.

Strategy (edge-sharded, dst-partitioned, gather-free):
  - Node dsts are range-partitioned over the 8 cores (6250 dsts/core); each core
    owns all edges into its dst range (~200K edges), sorted by dst and grouped
    into 128-dst chunks, padded to 128-edge tiles (pad slots marked dstcmp=-1).
  - The host replicates node features along edges (the "replicated node
    features" sharding): per core it stages x[src[e]]^T and x[dst[e]]^T as two
    big sequential fp32 streams. This avoids per-edge DMA gathers entirely
    (SWDGE descriptor generation is ~8ns/row and would dominate).
  - On device, per 128-edge tile:
      k_e|v_e = xsT_tile.T @ [Wk^T | Wv^T]   (PE, per-edge projection)
      q_e     = xdT_tile.T @ Wq^T            (PE)
      scores  = head-wise dot(k_e, q_e)      (DVE mul + grouped reduce)
      p       = exp(scores)                  (ACT; no max-subtraction needed:
                                              |scores| < ~40 so fp32 exp is
                                              safe, and softmax is shift-free)
      pv      = v_e * p                      (GPSIMD)
      per-dst-chunk segmented sums of [pv | p] via one-hot matmul (PE) into a
      PSUM accumulator; finalize out = pv_sum / p_sum per chunk, DMA out.
  - Output: concat of per-core [6250,128] blocks -> [50000,1,128].
"""
import math
import numpy as np

import concourse.bass as bass
import concourse.tile as tile
from concourse import bacc, mybir
from concourse.bass_utils import run_bass_kernel_spmd

f32 = mybir.dt.float32
f32r = mybir.dt.float32r
bf16 = mybir.dt.bfloat16

# problem shape (hardcoded per contract)
N = 50000
E = 1600000
D = 128
H = 4
NC = 8
NDST = N // NC          # 6250 dsts per core
CH = 128                # dsts per chunk
NCHUNK = (NDST + CH - 1) // CH   # 49
GT = 4                  # tiles per DVE/ACT batch group
TB = 128                # edges per tile


def _schedule(src, dst):
    """Sort/pad edges per core; return common tile counts + per-core slot arrays."""
    core = dst // NDST
    dstl = dst % NDST
    per_core = []
    counts = np.zeros((NC, NCHUNK), np.int64)
    for c in range(NC):
        sel = np.nonzero(core == c)[0]
        order = np.argsort(dstl[sel], kind="stable")
        e = sel[order]
        chunk = dstl[e] // CH
        counts[c] = np.bincount(chunk, minlength=NCHUNK)
        per_core.append((e, dstl[e]))
    T = np.maximum(1, np.ceil(counts.max(axis=0) / TB).astype(np.int64))  # [NCHUNK]
    NT = int(T.sum())
    ETOT = NT * TB
    tile_base = np.concatenate([[0], np.cumsum(T)])  # chunk -> first tile
    slots_src = np.zeros((NC, ETOT), np.int64)
    slots_dst = np.zeros((NC, ETOT), np.int64)
    dcmp = np.full((NC, ETOT), -1.0, np.float32)
    for c in range(NC):
        e, dl = per_core[c]
        pos = 0
        for j in range(NCHUNK):
            n = counts[c, j]
            base = int(tile_base[j]) * TB
            slots_src[c, base:base + n] = src[e[pos:pos + n]]
            slots_dst[c, base:base + n] = dst[e[pos:pos + n]]
            dcmp[c, base:base + n] = (dl[pos:pos + n] - j * CH).astype(np.float32)
            pos += n
    return T, tile_base, slots_src, slots_dst, dcmp


def _build(T, has_bias):
    """Emit the SPMD Bass program for common schedule T (tiles per chunk)."""
    T = [int(t) for t in T]
    NT = sum(T)
    ETOT = NT * TB
    nc = bacc.Bacc("TRN2", target_bir_lowering=False, debug=False, num_devices=NC)
    xsT = nc.dram_tensor("xsT", [128, ETOT], f32r, kind="ExternalInput").ap()
    xdT = nc.dram_tensor("xdT", [128, ETOT], f32r, kind="ExternalInput").ap()
    dcmp_d = nc.dram_tensor("dcmp", [128, NT, 1], f32, kind="ExternalInput").ap()
    wk_d = nc.dram_tensor("Wk", [128, 128], f32, kind="ExternalInput").ap()
    wq_d = nc.dram_tensor("Wq", [128, 128], f32, kind="ExternalInput").ap()
    wv_d = nc.dram_tensor("Wv", [128, 128], f32, kind="ExternalInput").ap()
    ident_d = nc.dram_tensor("ident", [128, 128], f32, kind="ExternalInput").ap()
    iota_d = nc.dram_tensor("iota4", [128, GT * 128], f32, kind="ExternalInput").ap()
    if has_bias:
        bkv_d = nc.dram_tensor("bkv", [1, 256], f32, kind="ExternalInput").ap()
        bq_d = nc.dram_tensor("bq", [1, 128], f32, kind="ExternalInput").ap()
        ones_d = nc.dram_tensor("ones", [1, 128], f32, kind="ExternalInput").ap()
    out_d = nc.dram_tensor("out", [NDST, 128], f32, kind="ExternalOutput").ap()

    with tile.TileContext(nc) as tc:
        with (
            tc.tile_pool(name="const", bufs=1) as cpool,
            tc.tile_pool(name="xs", bufs=3) as xspool,
            tc.tile_pool(name="xd", bufs=3) as xdpool,
            tc.tile_pool(name="kvsb", bufs=4) as kvsbpool,
            tc.tile_pool(name="work", bufs=3) as wpool,
            tc.tile_pool(name="dc", bufs=2) as dcpool,
            tc.tile_pool(name="kvps", bufs=2, space="PSUM") as kvpspool,
            tc.tile_pool(name="qps", bufs=1, space="PSUM") as qpspool,
            tc.tile_pool(name="aggps", bufs=2, space="PSUM") as aggpool,
        ):
            # ---- setup: load consts, transpose Wk/Wq/Wv on PE ----
            wk_sb = cpool.tile([128, 128], f32, tag="wk")
            wq_sb = cpool.tile([128, 128], f32, tag="wq")
            wv_sb = cpool.tile([128, 128], f32, tag="wv")
            ident = cpool.tile([128, 128], f32, tag="ident")
            iota4 = cpool.tile([128, GT * 128], f32, tag="iota4")
            nc.scalar.dma_start(out=wk_sb[:], in_=wk_d[:])
            nc.scalar.dma_start(out=wq_sb[:], in_=wq_d[:])
            nc.scalar.dma_start(out=wv_sb[:], in_=wv_d[:])
            nc.scalar.dma_start(out=ident[:], in_=ident_d[:])
            nc.scalar.dma_start(out=iota4[:], in_=iota_d[:])
            wkvT = cpool.tile([128, 256], f32r, tag="wkvT")  # [Wk^T | Wv^T] (f32r)
            wqT = cpool.tile([128, 256], f32r, tag="wqT")
            if has_bias:
                bkv_sb = cpool.tile([1, 256], f32, tag="bkv")
                bq_sb = cpool.tile([1, 128], f32, tag="bq")
                ones_sb = cpool.tile([1, 128], f32, tag="ones")
                nc.scalar.dma_start(out=bkv_sb[:], in_=bkv_d[:])
                nc.scalar.dma_start(out=bq_sb[:], in_=bq_d[:])
                nc.scalar.dma_start(out=ones_sb[:], in_=ones_d[:])
            for w_sb, dst_slices in ((wk_sb, [wkvT[:, 0:128]]),
                                     (wv_sb, [wkvT[:, 128:256]]),
                                     (wq_sb, [wqT[:, 0:128], wqT[:, 128:256]])):
                tp = aggpool.tile([128, 256, 1], f32, tag="agg")
                nc.tensor.transpose(out=tp[:, 0:128, 0], in_=w_sb[:], identity=ident[:])
                for ds in dst_slices:
                    nc.scalar.copy(out=ds, in_=tp[:, 0:128, 0])

            # ---- edge phase ----
            for j in range(NCHUNK):
                tbase = sum(T[:j])
                tj = T[j]
                agg = aggpool.tile([128, 256, 1], f32, tag="agg")
                dc_t = dcpool.tile([128, max(T), 1], f32, tag="dc")
                nc.sync.dma_start(out=dc_t[:, :tj, :],
                                    in_=dcmp_d[:, tbase:tbase + tj, :])
                # stream xs/xd in 16-tile (1MB) blocks for DMA efficiency
                blocks = []
                t0b = 0
                BLK = 16
                while t0b < tj:
                    L = min(BLK, tj - t0b)
                    xs_b = xspool.tile([128, BLK * TB], f32r, tag="xs")
                    xd_b = xdpool.tile([128, BLK * TB], f32r, tag="xd")
                    c0 = (tbase + t0b) * TB
                    nc.sync.dma_start(out=xs_b[:, : L * TB],
                                      in_=xsT[:, c0:c0 + L * TB])
                    nc.sync.dma_start(out=xd_b[:, : L * TB],
                                      in_=xdT[:, c0:c0 + L * TB])
                    blocks.append((xs_b, xd_b))
                    t0b += L
                done = 0
                while done < tj:
                    r = min(GT, tj - done)
                    xs_bt, xd_bt = blocks[done // BLK]
                    loc = (done % BLK) * TB
                    xs_t = xs_bt[:, loc:loc + r * TB]
                    xd_t = xd_bt[:, loc:loc + r * TB]
                    kv_ps = kvpspool.tile([128, GT, 256], f32, tag="kvps")
                    q_ps = qpspool.tile([128, GT, 256], f32, tag="qps")
                    for i in range(r):
                        sl = slice(i * 128, (i + 1) * 128)
                        nc.tensor.matmul(out=kv_ps[:, i, :], lhsT=xs_t[:, sl],
                                         rhs=wkvT[:], start=True, stop=not has_bias,
                                         skip_group_check=True)
                        if has_bias:
                            nc.tensor.matmul(out=kv_ps[:, i, :], lhsT=ones_sb[0:1, :],
                                             rhs=bkv_sb[0:1, :], start=False,
                                             stop=True, skip_group_check=True)
                        nc.tensor.matmul(out=q_ps[:, i, :], lhsT=xd_t[:, sl],
                                         rhs=wqT[:], start=True, stop=not has_bias,
                                         skip_group_check=True)
                        if has_bias:
                            nc.tensor.matmul(out=q_ps[:, i, 0:128], lhsT=ones_sb[0:1, :],
                                             rhs=bq_sb[0:1, :], start=False,
                                             stop=True, skip_group_check=True)
                    k_sb = kvsbpool.tile([128, GT, 128], f32, tag="ksb")
                    nc.scalar.copy(out=k_sb[:, :r, :], in_=kv_ps[:, :r, 0:128])
                    v_sb = kvsbpool.tile([128, GT, 128], f32r, tag="vsb")
                    nc.scalar.copy(out=v_sb[:, :r, :], in_=kv_ps[:, :r, 128:256])
                    sel_e = wpool.tile([128, GT, 128], f32r, tag="sel")
                    nc.vector.tensor_tensor(
                        out=sel_e[:, :r, :],
                        in0=dc_t[:, done:done + r, :].to_broadcast([128, r, 128]),
                        in1=iota4[:, : r * 128].rearrange("p (r c) -> p r c", r=r),
                        op=mybir.AluOpType.is_equal,
                    )
                    prod = wpool.tile([128, GT, H, 32], f32, tag="prod")
                    nc.vector.tensor_tensor(
                        out=prod[:, :r],
                        in0=k_sb[:, :r, :].rearrange("p r (h c) -> p r h c", h=H),
                        in1=q_ps[:, :r, 0:128].rearrange("p r (h c) -> p r h c", h=H),
                        op=mybir.AluOpType.mult,
                    )
                    scores = wpool.tile([128, GT, H], f32, tag="scores")
                    nc.vector.tensor_reduce(out=scores[:, :r, :], in_=prod[:, :r],
                                            axis=mybir.AxisListType.X,
                                            op=mybir.AluOpType.add)
                    pvp = wpool.tile([128, GT, 256, 1], f32r, tag="pvp")
                    nc.scalar.activation(out=pvp[:, :r, 128:132, 0],
                                         in_=scores[:, :r, :],
                                         func=mybir.ActivationFunctionType.Exp)
                    nc.gpsimd.tensor_tensor(
                        out=pvp[:, :r, 0:128, 0].rearrange("p r (h c) -> p r h c",
                                                           h=H),
                        in0=v_sb[:, :r, :].rearrange("p r (h c) -> p r h c", h=H),
                        in1=pvp[:, :r, 128:132, :].to_broadcast([128, r, H, 32]),
                        op=mybir.AluOpType.mult,
                    )
                    for i in range(r):
                        nc.tensor.matmul(out=agg[:, :, 0], lhsT=sel_e[:, i, :],
                                         rhs=pvp[:, i, :, 0],
                                         start=(done + i == 0),
                                         stop=(done + i == tj - 1),
                                         skip_group_check=True)
                    done += r
                # finalize chunk
                den = wpool.tile([128, H, 1], f32, tag="den")
                nc.vector.tensor_scalar_max(den[:], agg[:, 128:132, :], 1e-30)
                rec = wpool.tile([128, H, 1], f32, tag="rec")
                nc.vector.reciprocal(rec[:], den[:])
                outn = wpool.tile([128, H, 32], f32, tag="outn")
                nc.vector.tensor_tensor(
                    out=outn[:],
                    in0=agg[:, 0:128, 0].rearrange("p (h c) -> p h c", h=H),
                    in1=rec[:].to_broadcast([128, H, 32]),
                    op=mybir.AluOpType.mult,
                )
                rows = min(CH, NDST - j * CH)
                nc.sync.dma_start(
                    out=out_d[j * CH: j * CH + rows, :],
                    in_=outn[:rows].rearrange("p h c -> p (h c)"),
                )
    nc.compile()
    return nc


def kernel(**inputs):
    x = np.ascontiguousarray(np.asarray(inputs["x"], np.float32))
    Wk = np.ascontiguousarray(np.asarray(inputs["Wk"], np.float32))
    Wq = np.ascontiguousarray(np.asarray(inputs["Wq"], np.float32))
    Wv = np.ascontiguousarray(np.asarray(inputs["Wv"], np.float32))
    bk = np.asarray(inputs["bk"], np.float32)
    bq = np.asarray(inputs["bq"], np.float32)
    bv = np.asarray(inputs["bv"], np.float32)
    src = np.asarray(inputs["src"]).astype(np.int64)
    dst = np.asarray(inputs["dst"]).astype(np.int64)

    has_bias = bool(bk.any() or bq.any() or bv.any())
    T, tile_base, slots_src, slots_dst, dcmp = _schedule(src, dst)
    nc = _build(T, has_bias)

    ident = np.eye(128, dtype=np.float32)
    iota4 = np.tile(np.arange(128, dtype=np.float32), (128, GT))
    in_maps = []
    for c in range(NC):
        xs = np.ascontiguousarray(x[slots_src[c]].T)          # [128, ETOT]
        xd = np.ascontiguousarray(x[slots_dst[c]].T)
        m = {
            "xsT": xs,
            "xdT": xd,
            "dcmp": np.ascontiguousarray(dcmp[c].reshape(-1, TB).T)[:, :, None],
            "Wk": Wk, "Wq": Wq, "Wv": Wv,
            "ident": ident, "iota4": iota4,
        }
        if has_bias:
            m["bkv"] = np.concatenate([bk, bv]).reshape(1, 256).astype(np.float32)
            m["bq"] = bq.reshape(1, 128).astype(np.float32)
            m["ones"] = np.ones((1, 128), np.float32)
        in_maps.append(m)

    import os
    trace_dir = os.environ.get("BASS_GAT_TRACE")
    kw = {}
    if trace_dir:
        os.makedirs(trace_dir, exist_ok=True)
        kw = dict(trace=True, tmpdir=trace_dir)
    res = run_bass_kernel_spmd(nc, in_maps, core_ids=list(range(NC)), **kw)
    if trace_dir and res.exec_time_ns is not None:
        print(f"HW exec time: {res.exec_time_ns} ns")
    out = np.concatenate([res.results[c]["out"] for c in range(NC)], axis=0)
    return out.reshape(N, 1, D).astype(np.float32)


if __name__ == "__main__":
    # tiny self-check with random data against a numpy reference
    rng = np.random.default_rng(0)
    ins = {
        "x": rng.standard_normal((N, D), np.float32),
        "Wk": (rng.standard_normal((D, D)) / math.sqrt(D)).astype(np.float32),
        "bk": np.zeros(D, np.float32),
        "Wq": (rng.standard_normal((D, D)) / math.sqrt(D)).astype(np.float32),
        "bq": np.zeros(D, np.float32),
        "Wv": (rng.standard_normal((D, D)) / math.sqrt(D)).astype(np.float32),
        "bv": np.zeros(D, np.float32),
        "src": rng.integers(0, N, E).astype(np.int32),
        "dst": rng.integers(0, N, E).astype(np.int32),
    }
    out = kernel(**ins)
    print("out", out.shape, out.dtype, np.abs(out).max())


# revision 11
# speedup vs baseline: 1.5790x; 1.0527x over previous
"""GAT layer (nn_GATLayer_32719060861314) as a Bass/Tile SPMD kernel on 8 trn2 cores.

Strategy (edge-sharded, dst-partitioned, gather-free):
  - Node dsts are range-partitioned over the 8 cores (6250 dsts/core); each core
    owns all edges into its dst range (~200K edges), sorted by dst and grouped
    into 128-dst chunks, padded to 128-edge tiles (pad slots get an all-zero
    one-hot column, so they contribute nothing).
  - The host replicates node features along edges (the "replicated node
    features" sharding): per core it stages x[src[e]]^T as one big sequential
    f32r stream, plus a bf16 one-hot stream sel_dst[w,e] = (dst_local[e]==w)
    per 128-dst chunk. This avoids per-edge DMA gathers entirely (SWDGE
    descriptor generation is ~8ns/row and would dominate).
  - On device, per 128-edge tile:
      k_e|v_e = xsT_tile.T @ [Wk^T | Wv^T]    (PE, f32r single-pass)
      q_chunk = xq_chunk^T.T @ Wq^T           (PE, fp32, once per 128-dst chunk,
                                               split into bf16 hi+lo)
      q_e     = sel_dst.T @ q_hi + sel_dst.T @ q_lo   (PE, bf16, PSUM-summed)
      scores  = head-wise dot(k_e, q_e)       (DVE mul + grouped reduce)
      p       = exp(scores)                   (ACT; no max-subtraction needed:
                                               |scores| < ~40 so fp32 exp is
                                               safe, and softmax is shift-free)
      pv      = v_e * p                       (GPSIMD)
      per-dst-chunk segmented sums of [pv | p] via one-hot matmul (PE,
      lhsT = sel_e built on DVE) accumulated in PSUM; finalize
      out = pv_sum / p_sum per chunk, DMA out.
  - Output: concat of per-core [6250,128] blocks -> [50000,1,128].
"""
import math
import numpy as np
import ml_dtypes

import concourse.bass as bass
import concourse.tile as tile
from concourse import bacc, mybir
from concourse.bass_utils import run_bass_kernel_spmd

f32 = mybir.dt.float32
f32r = mybir.dt.float32r
bf16 = mybir.dt.bfloat16

# problem shape (hardcoded per contract)
N = 50000
E = 1600000
D = 128
H = 4
NC = 8
NDST = N // NC          # 6250 dsts per core
CH = 128                # dsts per chunk
NCHUNK = (NDST + CH - 1) // CH   # 49
GT = 4                  # tiles per DVE/ACT batch group
TB = 128                # edges per tile
BLK = 16                # tiles per stream block (1MB loads)


def _schedule(src, dst):
    """Sort/pad edges per core; return common tile counts + per-core slot arrays."""
    core = dst // NDST
    dstl = dst % NDST
    per_core = []
    counts = np.zeros((NC, NCHUNK), np.int64)
    for c in range(NC):
        sel = np.nonzero(core == c)[0]
        order = np.argsort(dstl[sel], kind="stable")
        e = sel[order]
        chunk = dstl[e] // CH
        counts[c] = np.bincount(chunk, minlength=NCHUNK)
        per_core.append((e, dstl[e]))
    T = np.maximum(1, np.ceil(counts.max(axis=0) / TB).astype(np.int64))  # [NCHUNK]
    NT = int(T.sum())
    ETOT = NT * TB
    tile_base = np.concatenate([[0], np.cumsum(T)])  # chunk -> first tile
    slots_src = np.zeros((NC, ETOT), np.int64)
    dcmp = np.full((NC, ETOT), -1.0, np.float32)
    for c in range(NC):
        e, dl = per_core[c]
        pos = 0
        for j in range(NCHUNK):
            n = counts[c, j]
            base = int(tile_base[j]) * TB
            slots_src[c, base:base + n] = src[e[pos:pos + n]]
            dcmp[c, base:base + n] = (dl[pos:pos + n] - j * CH).astype(np.float32)
            pos += n
    return T, slots_src, dcmp


def _build(T, has_bias):
    """Emit the SPMD Bass program for common schedule T (tiles per chunk)."""
    T = [int(t) for t in T]
    NT = sum(T)
    ETOT = NT * TB
    NDSTP = NCHUNK * CH
    nc = bacc.Bacc("TRN2", target_bir_lowering=False, debug=False, num_devices=NC)
    xsT = nc.dram_tensor("xsT", [128, ETOT], f32r, kind="ExternalInput").ap()
    sds_d = nc.dram_tensor("seldst", [128, ETOT], bf16, kind="ExternalInput").ap()
    xqT_d = nc.dram_tensor("xqT", [128, NDSTP], f32, kind="ExternalInput").ap()
    dcmp_d = nc.dram_tensor("dcmp", [128, NT, 1], f32, kind="ExternalInput").ap()
    wk_d = nc.dram_tensor("Wk", [128, 128], f32, kind="ExternalInput").ap()
    wq_d = nc.dram_tensor("Wq", [128, 128], f32, kind="ExternalInput").ap()
    wv_d = nc.dram_tensor("Wv", [128, 128], f32, kind="ExternalInput").ap()
    ident_d = nc.dram_tensor("ident", [128, 128], f32, kind="ExternalInput").ap()
    iota_d = nc.dram_tensor("iotab", [128, BLK * 128], f32, kind="ExternalInput").ap()
    if has_bias:
        bkv_d = nc.dram_tensor("bkv", [1, 256], f32, kind="ExternalInput").ap()
        bq_d = nc.dram_tensor("bq", [1, 128], f32, kind="ExternalInput").ap()
        ones_d = nc.dram_tensor("ones", [1, 128], f32, kind="ExternalInput").ap()
    out_d = nc.dram_tensor("out", [NDST, 128], f32, kind="ExternalOutput").ap()

    with tile.TileContext(nc) as tc:
        with (
            tc.tile_pool(name="const", bufs=1) as cpool,
            tc.tile_pool(name="xs", bufs=3) as xspool,
            tc.tile_pool(name="sds", bufs=3) as sdspool,
            tc.tile_pool(name="kvsb", bufs=4) as kvsbpool,
            tc.tile_pool(name="work", bufs=3) as wpool,
            tc.tile_pool(name="selp", bufs=2) as selpool,
            tc.tile_pool(name="qc", bufs=2) as qcpool,
            tc.tile_pool(name="dc", bufs=2) as dcpool,
            tc.tile_pool(name="kvps", bufs=2, space="PSUM") as kvpspool,
            tc.tile_pool(name="qps", bufs=2, space="PSUM") as qpspool,
            tc.tile_pool(name="aggps", bufs=2, space="PSUM") as aggpool,
        ):
            # ---- setup: load consts, transpose Wk/Wq/Wv on PE ----
            wk_sb = cpool.tile([128, 128], f32, tag="wk")
            wq_sb = cpool.tile([128, 128], f32, tag="wq")
            wv_sb = cpool.tile([128, 128], f32, tag="wv")
            ident = cpool.tile([128, 128], f32, tag="ident")
            iotab = cpool.tile([128, BLK * 128], f32, tag="iotab")
            nc.scalar.dma_start(out=wk_sb[:], in_=wk_d[:])
            nc.scalar.dma_start(out=wq_sb[:], in_=wq_d[:])
            nc.scalar.dma_start(out=wv_sb[:], in_=wv_d[:])
            nc.scalar.dma_start(out=ident[:], in_=ident_d[:])
            nc.scalar.dma_start(out=iotab[:], in_=iota_d[:])
            wkvT = cpool.tile([128, 256], f32r, tag="wkvT")  # [Wk^T | Wv^T] (f32r)
            wqT = cpool.tile([128, 128], f32, tag="wqT")
            if has_bias:
                bkv_sb = cpool.tile([1, 256], f32, tag="bkv")
                bq_sb = cpool.tile([1, 128], f32, tag="bq")
                ones_sb = cpool.tile([1, 128], f32, tag="ones")
                nc.scalar.dma_start(out=bkv_sb[:], in_=bkv_d[:])
                nc.scalar.dma_start(out=bq_sb[:], in_=bq_d[:])
                nc.scalar.dma_start(out=ones_sb[:], in_=ones_d[:])
            for w_sb, dst_slice in ((wk_sb, wkvT[:, 0:128]),
                                    (wv_sb, wkvT[:, 128:256]),
                                    (wq_sb, wqT[:])):
                tp = aggpool.tile([128, 256, 1], f32, tag="agg")
                nc.tensor.transpose(out=tp[:, 0:128, 0], in_=w_sb[:], identity=ident[:])
                nc.scalar.copy(out=dst_slice, in_=tp[:, 0:128, 0])

            # ---- edge phase ----
            for j in range(NCHUNK):
                tbase = sum(T[:j])
                tj = T[j]
                agg = aggpool.tile([128, 256, 1], f32, tag="agg")
                dc_t = dcpool.tile([128, max(T), 1], f32, tag="dc")
                nc.sync.dma_start(out=dc_t[:, :tj, :],
                                  in_=dcmp_d[:, tbase:tbase + tj, :])

                # per-chunk q: q_chunk = xq_chunk @ Wq^T (fp32), split bf16 hi/lo
                qc_ps = qpspool.tile([128, GT, 128], f32, tag="qps")
                xq_t = qcpool.tile([128, 128], f32, tag="xqt")
                nc.sync.dma_start(out=xq_t[:],
                                  in_=xqT_d[:, j * CH:(j + 1) * CH])
                nc.tensor.matmul(out=qc_ps[:, 0, :], lhsT=xq_t[:], rhs=wqT[:],
                                 start=True, stop=not has_bias,
                                 skip_group_check=True)
                if has_bias:
                    nc.tensor.matmul(out=qc_ps[:, 0, :], lhsT=ones_sb[0:1, :],
                                     rhs=bq_sb[0:1, :], start=False, stop=True,
                                     skip_group_check=True)
                qhi = qcpool.tile([128, 128], bf16, tag="qhi")
                qhi32 = qcpool.tile([128, 128], f32, tag="qhi32")
                qlo = qcpool.tile([128, 128], bf16, tag="qlo")
                nc.scalar.copy(out=qhi[:], in_=qc_ps[:, 0, :])
                nc.scalar.copy(out=qhi32[:], in_=qhi[:])
                nc.vector.tensor_tensor(out=qlo[:], in0=qc_ps[:, 0, :],
                                        in1=qhi32[:], op=mybir.AluOpType.subtract)

                # stream xs / sel_dst in 16-tile blocks; build sel_e per block
                blocks = []
                t0b = 0
                while t0b < tj:
                    L = min(BLK, tj - t0b)
                    xs_b = xspool.tile([128, BLK * TB], f32r, tag="xs")
                    sds_b = sdspool.tile([128, BLK * TB], bf16, tag="sds")
                    c0 = (tbase + t0b) * TB
                    nc.sync.dma_start(out=xs_b[:, : L * TB],
                                      in_=xsT[:, c0:c0 + L * TB])
                    nc.sync.dma_start(out=sds_b[:, : L * TB],
                                      in_=sds_d[:, c0:c0 + L * TB])
                    sel_b = selpool.tile([128, BLK, 128], f32r, tag="sel")
                    nc.vector.tensor_tensor(
                        out=sel_b[:, :L, :],
                        in0=dc_t[:, t0b:t0b + L, :].to_broadcast([128, L, 128]),
                        in1=iotab[:, : L * 128].rearrange("p (t c) -> p t c", t=L),
                        op=mybir.AluOpType.is_equal,
                    )
                    blocks.append((xs_b, sds_b, sel_b))
                    t0b += L

                done = 0
                while done < tj:
                    r = min(GT, tj - done)
                    xs_bt, sds_bt, sel_bt = blocks[done // BLK]
                    loc = (done % BLK) * TB
                    kv_ps = kvpspool.tile([128, GT, 256], f32, tag="kvps")
                    q_ps = qpspool.tile([128, GT, 128], f32, tag="qps")
                    for i in range(r):
                        sl = slice(loc + i * TB, loc + (i + 1) * TB)
                        nc.tensor.matmul(out=kv_ps[:, i, :], lhsT=xs_bt[:, sl],
                                         rhs=wkvT[:], start=True, stop=not has_bias,
                                         skip_group_check=True)
                        if has_bias:
                            nc.tensor.matmul(out=kv_ps[:, i, :], lhsT=ones_sb[0:1, :],
                                             rhs=bkv_sb[0:1, :], start=False,
                                             stop=True, skip_group_check=True)
                        nc.tensor.matmul(out=q_ps[:, i, :], lhsT=sds_bt[:, sl],
                                         rhs=qhi[:], start=True, stop=False,
                                         skip_group_check=True)
                        nc.tensor.matmul(out=q_ps[:, i, :], lhsT=sds_bt[:, sl],
                                         rhs=qlo[:], start=False, stop=True,
                                         skip_group_check=True)
                    kv_sb = kvsbpool.tile([128, GT, 256], f32r, tag="kvsb")
                    nc.scalar.copy(out=kv_sb[:, :r, :], in_=kv_ps[:, :r, :])
                    prod = wpool.tile([128, GT, H, 32], f32, tag="prod")
                    nc.vector.tensor_tensor(
                        out=prod[:, :r],
                        in0=kv_sb[:, :r, 0:128].bitcast(f32)
                            .rearrange("p r (h c) -> p r h c", h=H),
                        in1=q_ps[:, :r, :].rearrange("p r (h c) -> p r h c", h=H),
                        op=mybir.AluOpType.mult,
                    )
                    scores = wpool.tile([128, GT, H], f32, tag="scores")
                    nc.vector.tensor_reduce(out=scores[:, :r, :], in_=prod[:, :r],
                                            axis=mybir.AxisListType.X,
                                            op=mybir.AluOpType.add)
                    pvp = wpool.tile([128, GT, 256, 1], f32r, tag="pvp")
                    nc.scalar.activation(out=pvp[:, :r, 128:132, 0],
                                         in_=scores[:, :r, :],
                                         func=mybir.ActivationFunctionType.Exp)
                    nc.gpsimd.tensor_tensor(
                        out=pvp[:, :r, 0:128, 0].rearrange("p r (h c) -> p r h c",
                                                           h=H),
                        in0=kv_sb[:, :r, 128:256].rearrange("p r (h c) -> p r h c",
                                                            h=H),
                        in1=pvp[:, :r, 128:132, :].to_broadcast([128, r, H, 32]),
                        op=mybir.AluOpType.mult,
                    )
                    for i in range(r):
                        nc.tensor.matmul(out=agg[:, :, 0],
                                         lhsT=sel_bt[:, (done % BLK) + i, :],
                                         rhs=pvp[:, i, :, 0],
                                         start=(done + i == 0),
                                         stop=(done + i == tj - 1),
                                         skip_group_check=True)
                    done += r
                # finalize chunk
                den = wpool.tile([128, H, 1], f32, tag="den")
                nc.vector.tensor_scalar_max(den[:], agg[:, 128:132, :], 1e-30)
                rec = wpool.tile([128, H, 1], f32, tag="rec")
                nc.vector.reciprocal(rec[:], den[:])
                outn = wpool.tile([128, H, 32], f32, tag="outn")
                nc.vector.tensor_tensor(
                    out=outn[:],
                    in0=agg[:, 0:128, 0].rearrange("p (h c) -> p h c", h=H),
                    in1=rec[:].to_broadcast([128, H, 32]),
                    op=mybir.AluOpType.mult,
                )
                rows = min(CH, NDST - j * CH)
                nc.sync.dma_start(
                    out=out_d[j * CH: j * CH + rows, :],
                    in_=outn[:rows].rearrange("p h c -> p (h c)"),
                )
    nc.compile()
    return nc


def kernel(**inputs):
    x = np.ascontiguousarray(np.asarray(inputs["x"], np.float32))
    Wk = np.ascontiguousarray(np.asarray(inputs["Wk"], np.float32))
    Wq = np.ascontiguousarray(np.asarray(inputs["Wq"], np.float32))
    Wv = np.ascontiguousarray(np.asarray(inputs["Wv"], np.float32))
    bk = np.asarray(inputs["bk"], np.float32)
    bq = np.asarray(inputs["bq"], np.float32)
    bv = np.asarray(inputs["bv"], np.float32)
    src = np.asarray(inputs["src"]).astype(np.int64)
    dst = np.asarray(inputs["dst"]).astype(np.int64)

    has_bias = bool(bk.any() or bq.any() or bv.any())
    T, slots_src, dcmp = _schedule(src, dst)
    nc = _build(T, has_bias)

    NDSTP = NCHUNK * CH
    ETOT = slots_src.shape[1]
    ident = np.eye(128, dtype=np.float32)
    iotab = np.tile(np.arange(128, dtype=np.float32), (128, BLK))
    in_maps = []
    for c in range(NC):
        xs = np.ascontiguousarray(x[slots_src[c]].T)          # [128, ETOT]
        sds = np.zeros((128, ETOT), ml_dtypes.bfloat16)
        cols = np.nonzero(dcmp[c] >= 0)[0]
        sds[dcmp[c][cols].astype(np.int64), cols] = 1
        xq = np.zeros((128, NDSTP), np.float32)
        xq[:, :NDST] = x[c * NDST:(c + 1) * NDST].T
        m = {
            "xsT": xs,
            "seldst": sds,
            "xqT": xq,
            "dcmp": np.ascontiguousarray(dcmp[c].reshape(-1, TB).T)[:, :, None],
            "Wk": Wk, "Wq": Wq, "Wv": Wv,
            "ident": ident, "iotab": iotab,
        }
        if has_bias:
            m["bkv"] = np.concatenate([bk, bv]).reshape(1, 256).astype(np.float32)
            m["bq"] = bq.reshape(1, 128).astype(np.float32)
            m["ones"] = np.ones((1, 128), np.float32)
        in_maps.append(m)

    import os
    trace_dir = os.environ.get("BASS_GAT_TRACE")
    kw = {}
    if trace_dir:
        os.makedirs(trace_dir, exist_ok=True)
        kw = dict(trace=True, tmpdir=trace_dir)
    res = run_bass_kernel_spmd(nc, in_maps, core_ids=list(range(NC)), **kw)
    if trace_dir and res.exec_time_ns is not None:
        print(f"HW exec time: {res.exec_time_ns} ns")
    out = np.concatenate([res.results[c]["out"] for c in range(NC)], axis=0)
    return out.reshape(N, 1, D).astype(np.float32)


if __name__ == "__main__":
    rng = np.random.default_rng(0)
    ins = {
        "x": rng.standard_normal((N, D), np.float32),
        "Wk": (rng.standard_normal((D, D)) / math.sqrt(D)).astype(np.float32),
        "bk": np.zeros(D, np.float32),
        "Wq": (rng.standard_normal((D, D)) / math.sqrt(D)).astype(np.float32),
        "bq": np.zeros(D, np.float32),
        "Wv": (rng.standard_normal((D, D)) / math.sqrt(D)).astype(np.float32),
        "bv": np.zeros(D, np.float32),
        "src": rng.integers(0, N, E).astype(np.int32),
        "dst": rng.integers(0, N, E).astype(np.int32),
    }
    out = kernel(**ins)
    print("out", out.shape, out.dtype, np.abs(out).max())
